# revision 20
# baseline (speedup 1.0000x reference)
"""Trainium2 Bass kernel for nn_BandFunctionalPrior.

Math reduction: the reference embeds scalar x into d_embed via per-band
Linear(1,d), projects to q/k (d_s), and takes masked softmax of q@k^T.
Because e = x*W_e + b_e is affine in the scalar x, the score matrix
collapses to rank-1 plus j-constant terms that cancel in softmax:

    s[t,n,i,j] ~ u_i * x_j  (+ const_i terms, cancelled)
    u = (alpha_n * x + gamma_n) / sqrt(d_s)
    alpha_n = sum_s (W_e[n]@Wq^T)_s (W_e[n]@Wk^T)_s
    gamma_n = sum_s (b_e[n]@Wq^T)_s (W_e[n]@Wk^T)_s

So per (batch,t,band) the kernel computes exp(outer(u,x) + maskbias),
row-normalizes, weights by w = active_alpha*active_mask and sums over
bands.  Sharding: pure data parallel over batch (4 per core).

Device geometry per (b, band): 128 t-values -> 8 tile-groups of 16;
each PE matmul builds 16 stacked 64x64 score matrices as a [128, 512]
PSUM tile via a block-diagonal K=16 matmul, with the adjacency mask
bias added by a preceding constant matmul accumulating into the same
PSUM bank.  ACT does exp, DVE does row sums / reciprocal / weighting /
band accumulation.
"""

import sys

for _p in ("/opt/trn_rl_repo",):
    if _p not in sys.path:
        sys.path.insert(0, _p)

import numpy as np

NCORES = 8
B4 = 4          # batches per core
NB = 5          # bands
C = 64          # electrodes
T = 128         # time steps
TG = 8          # tile groups per (b, band); 16 t's each
NEG = -30000.0  # mask bias; exp(s + NEG) == 0 in fp32

_CACHE = {}


def _build_program():
    import concourse.bacc as bacc
    import concourse.mybir as mybir
    from concourse.tile import TileContext

    fp32 = mybir.dt.float32
    ALU = mybir.AluOpType
    ACTF = mybir.ActivationFunctionType
    AXIS = mybir.AxisListType

    nc = bacc.Bacc(None, target_bir_lowering=False)

    xt = nc.dram_tensor("xt", (B4, NB, T, C), fp32, kind="ExternalInput")
    ut = nc.dram_tensor("ut", (B4, NB, T, C), fp32, kind="ExternalInput")
    xo = nc.dram_tensor("xo", (B4, NB, C, T), fp32, kind="ExternalInput")
    wh = nc.dram_tensor("wh", (B4, NB, 2, C), fp32, kind="ExternalInput")
    md = nc.dram_tensor("md", (B4, T, NB), fp32, kind="ExternalInput")
    mb8 = nc.dram_tensor("mb8", (C, TG * C), fp32, kind="ExternalInput")
    i2 = nc.dram_tensor("i2", (C, 2 * C), fp32, kind="ExternalInput")
    wemb = nc.dram_tensor("wemb", (2 * NB, C), fp32, kind="ExternalInput")
    ones64 = nc.dram_tensor("ones64", (C, 1), fp32, kind="ExternalInput")
    id128 = nc.dram_tensor("id128", (T, T), fp32, kind="ExternalInput")
    pfunc = nc.dram_tensor("pfunc", (B4, T, C, C), fp32, kind="ExternalOutput")
    emb = nc.dram_tensor("emb", (B4, T, C), fp32, kind="ExternalOutput")

    with TileContext(nc) as tc:
        with (
            tc.tile_pool(name="const", bufs=1) as constp,
            tc.tile_pool(name="banks", bufs=1) as bankp,
            tc.tile_pool(name="work", bufs=6) as workp,
            tc.tile_pool(name="paccp", bufs=TG + 1) as paccp,
            tc.tile_pool(name="small", bufs=8) as smallp,
            tc.tile_pool(name="psum", bufs=4, space="PSUM") as psump,
            tc.tile_pool(name="psmall", bufs=1, space="PSUM") as psmallp,
        ):
            # ---- constants ----
            mb8_sb = constp.tile([C, TG * C], fp32)
            nc.sync.dma_start(mb8_sb, mb8[:, :])
            i2_sb = constp.tile([C, 2 * C], fp32)
            nc.sync.dma_start(i2_sb, i2[:, :])
            wemb_sb = constp.tile([2 * NB, C], fp32)
            nc.sync.dma_start(wemb_sb, wemb[:, :])
            ones_sb = constp.tile([C, 1], fp32)
            nc.sync.dma_start(ones_sb, ones64[:, :])
            id128_sb = constp.tile([T, T], fp32)
            nc.sync.dma_start(id128_sb, id128[:, :])


            # ---- persistent zero-padded u banks (ping-pong) ----
            # ubank2: [2, 8m * 8tg * 128]; pair (m, tg) occupies free slice
            # [(m*TG+tg)*128, +128): row 0 = u(t_even) at [0,64), row 1 =
            # u(t_odd) at [64,128), zeros elsewhere (K=2 h-block-diagonal
            # lhsT for two stacked 64-row outer products).
            ubanks = []
            xbanks = []
            for kk in range(2):
                ubk = bankp.tile([2, 8 * TG * 128], fp32, name=f"ubank{kk}")
                nc.vector.memset(ubk[:, :], 0.0)
                ubanks.append(ubk)
                # xbank: dense pair rows, pair (m, tg) at [(m*TG+tg)*64, +64)
                xbk = bankp.tile([2, 8 * TG * C], fp32, name=f"xbank{kk}")
                xbanks.append(xbk)

            for b in range(B4):
                # ---------- embeddings path ----------
                # xbar columns: [128t, 1] per band via matmul(lhsT=xo, rhs=ones)
                xbars = psmallp.tile([T, NB], fp32, tag="xbars")
                for n in range(NB):
                    xo_sb = workp.tile([C, T], fp32, tag="xo")
                    nc.sync.dma_start(xo_sb, xo[b, n])
                    nc.tensor.matmul(
                        xbars[:, n : n + 1], xo_sb, ones_sb, start=True, stop=True
                    )
                mdv_sb = smallp.tile([T, NB], fp32, tag="mdv")
                nc.sync.dma_start(mdv_sb, md[b])
                coef = smallp.tile([T, 2 * NB], fp32, tag="coef")
                nc.vector.tensor_mul(coef[:, :NB], xbars[:, :], mdv_sb[:, :])
                nc.vector.tensor_copy(coef[:, NB:], mdv_sb[:, :])
                coefT_ps = psmallp.tile([2 * NB, T], fp32, tag="coefT")
                nc.tensor.transpose(coefT_ps, coef, id128_sb)
                coefT_sb = smallp.tile([2 * NB, T], fp32, tag="coefTs")
                nc.vector.tensor_copy(coefT_sb, coefT_ps)
                emb_ps = psmallp.tile([T, C], fp32, tag="embp")
                nc.tensor.matmul(emb_ps, coefT_sb, wemb_sb, start=True, stop=True)
                emb_sb = smallp.tile([T, C], fp32, tag="embs")
                nc.vector.tensor_copy(emb_sb, emb_ps)
                nc.sync.dma_start(emb[b], emb_sb)

                # ---------- attention path ----------
                pacc = [
                    paccp.tile([128, 512], fp32, tag="pacc", name=f"pacc_{b}_{tg}")
                    for tg in range(TG)
                ]
                for n in range(NB):
                    kk = (b * NB + n) % 2
                    ubk, xbk = ubanks[kk], xbanks[kk]
                    # fill u/x banks: 2 DMAs each (h = 0, 1); src is flat DRAM
                    # u(t = tg*16 + 2m + h) -> free (m*TG + tg)*128 + h*64
                    # x(t)                  -> free (m*TG + tg)*64
                    ubv = ubk.rearrange(
                        "p (m tg c2 i) -> p m tg c2 i", m=8, tg=TG, c2=2, i=C
                    )
                    xbv = xbk.rearrange("p (m tg j) -> p m tg j", m=8, tg=TG, j=C)
                    usv = ut[b, n].rearrange(
                        "(tg m2 h2) i -> h2 m2 tg i", tg=TG, m2=8, h2=2
                    )
                    xsv = xt[b, n].rearrange(
                        "(tg m2 h2) j -> h2 m2 tg j", tg=TG, m2=8, h2=2
                    )
                    # w tile: [128=(h,i), (tg,m)] = w(t=tg*16+2m+h), bcast over i
                    wt = smallp.tile([T, C], fp32, tag="wt")
                    for h in range(2):
                        nc.sync.dma_start(ubv[h : h + 1, :, :, h], usv[h][None])
                        nc.sync.dma_start(xbv[h : h + 1], xsv[h][None])
                        nc.sync.dma_start(
                            wt[h * C : (h + 1) * C, :],
                            wh[b, n, h][None].to_broadcast((C, C)),
                        )
                    for tg in range(TG):
                        ps = psump.tile([128, 512], fp32, tag="ps")
                        nc.tensor.matmul(ps, i2_sb, mb8_sb, start=True, stop=False)
                        for m in range(8):
                            pr = m * TG + tg
                            nc.tensor.matmul(
                                ps[:, m * C : (m + 1) * C],
                                ubk[:, pr * 128 : (pr + 1) * 128],
                                xbk[:, pr * C : (pr + 1) * C],
                                start=False,
                                stop=True,
                            )
                        F = workp.tile([128, 512], fp32, tag="F")
                        nc.scalar.activation(F, ps, ACTF.Exp)
                        Fv = F.rearrange("p (m j) -> p m j", j=C)
                        Z = smallp.tile([128, TG], fp32, tag="Z")
                        nc.vector.tensor_reduce(
                            Z, Fv, axis=AXIS.X, op=ALU.add
                        )
                        r = smallp.tile([128, TG], fp32, tag="r")
                        nc.vector.reciprocal(r, Z)
                        r2 = smallp.tile([128, TG], fp32, tag="r2")
                        nc.vector.tensor_mul(
                            r2, r, wt[:, tg * 8 : (tg + 1) * 8]
                        )
                        rb = r2[:, :, None].broadcast_to((128, TG, C))
                        pv = pacc[tg].rearrange("p (m j) -> p m j", j=C)
                        if n == 0:
                            nc.vector.tensor_mul(pv, rb, Fv)
                        else:
                            tmp = workp.tile([128, 512], fp32, tag="tmp")
                            tv = tmp.rearrange("p (m j) -> p m j", j=C)
                            nc.vector.tensor_mul(tv, rb, Fv)
                            nc.vector.tensor_add(pacc[tg], pacc[tg], tmp)
                        if n == NB - 1:
                            dview = pfunc[b].rearrange(
                                "(tg m2 h) i j -> tg h i m2 j", tg=TG, m2=8, h=2
                            )
                            for h in range(2):
                                sview = pacc[tg][h * C : (h + 1) * C, :].rearrange(
                                    "i (m j) -> i m j", j=C
                                )
                                nc.sync.dma_start(dview[tg, h], sview)
    nc.finalize()
    return nc


def _get_program():
    if "nc" not in _CACHE:
        _CACHE["nc"] = _build_program()
    return _CACHE["nc"]


def kernel(x, adj, active_mask, active_alpha, W_e, b_e, Wq, Wk):
    x = np.asarray(x, np.float32)
    adj = np.asarray(adj, np.float32)
    active_mask = np.asarray(active_mask, np.float32)
    active_alpha = np.asarray(active_alpha, np.float32)
    W_e = np.asarray(W_e, np.float32)
    b_e = np.asarray(b_e, np.float32)
    Wq = np.asarray(Wq, np.float32)
    Wk = np.asarray(Wk, np.float32)

    ds = Wq.shape[0]
    Aq = W_e @ Wq.T
    Ak = W_e @ Wk.T
    Bq = b_e @ Wq.T
    a = (Aq * Ak).sum(-1) / np.sqrt(ds)
    g = (Bq * Ak).sum(-1) / np.sqrt(ds)

    xt = np.ascontiguousarray(x.transpose(0, 1, 3, 2))          # (32, 5, 128, 64)
    utf = (a[None, :, None, None] * xt + g[None, :, None, None]).astype(np.float32)
    w = (active_alpha * active_mask).astype(np.float32)          # (32, 128, 5)
    cnt = np.clip(active_mask.sum(-1, keepdims=True), 1.0, None)
    mdv = (active_mask / cnt).astype(np.float32)                 # (32, 128, 5)
    mb = np.where(adj > 0, 0.0, NEG).astype(np.float32)
    mb8_np = np.ascontiguousarray(np.tile(mb, (1, TG)))
    i2_np = np.ascontiguousarray(
        np.concatenate([np.eye(C, dtype=np.float32)] * 2, axis=1)
    )
    wemb_np = np.ascontiguousarray(
        np.concatenate([W_e / float(C), b_e], axis=0)
    ).astype(np.float32)
    ones_np = np.ones((C, 1), np.float32)
    id128_np = np.eye(T, dtype=np.float32)

    # w rearranged for the (h, i)-partition tiles: wh[b, n, h, tg*8+m]
    # = w[b, t=tg*16+2m+h, n]
    k = np.arange(C)
    whost = np.empty((32, NB, 2, C), np.float32)
    for h in range(2):
        tidx = (k // 8) * 16 + (k % 8) * 2 + h
        whost[:, :, h, :] = w[:, tidx, :].transpose(0, 2, 1)

    nc = _get_program()
    in_maps = []
    for c in range(NCORES):
        bs = slice(B4 * c, B4 * (c + 1))
        in_maps.append(
            dict(
                xt=np.ascontiguousarray(xt[bs]),
                ut=np.ascontiguousarray(utf[bs]),
                xo=np.ascontiguousarray(x[bs]),
                wh=np.ascontiguousarray(whost[bs]),
                md=np.ascontiguousarray(mdv[bs]),
                mb8=mb8_np,
                i2=i2_np,
                wemb=wemb_np,
                ones64=ones_np,
                id128=id128_np,
            )
        )

    from concourse.bass_utils import run_bass_kernel_spmd

    res = run_bass_kernel_spmd(nc, in_maps, core_ids=list(range(NCORES)))
    _CACHE["last_results"] = res
    P = np.concatenate([r["pfunc"] for r in res.results], axis=0)
    E = np.concatenate([r["emb"] for r in res.results], axis=0)
    return P, E
